# revision 8
# baseline (speedup 1.0000x reference)
"""Trainium2 Bass kernel for a ChannelAttention module.

Reference computation (per row b of B = 2048 rows, each row is (n=64, c=512)):
    y  = mean_c x                      # (B, 64)
    lr = y @ w1.T + b1                 # (B, 32)
    f1 = lr @ mb                       # (B, 128)
    at = softmax(f1 / sqrt(32))        # (B, 128)
    y1 = at @ mb.T                     # (B, 32)
    y2 = sigmoid(y1 @ w2.T + b2)       # (B, 64)
    out = x * y2[..., None]

Memory-bound: 256 MiB in + 256 MiB out. Strategy: data-parallel over 8 cores
(256 rows each), single streaming pass per core. The two inner linears fold
host-side into two small fused matrices so the on-chip MLP is:
    f1_raw = y_sum @ A          A = (w1.T @ mb) / 512          [64, 128]
    e      = exp(f1_raw*s + be) be = (b1 @ mb) * s, s=32^-0.5  [128, 1]
    [z|S]  = Daug.T @ e         Daug = [(w2 @ mb).T | ones]    [128, 65]
    y2     = sigmoid(z / S + b2)
(softmax max-subtraction is skipped: |f1*s| < ~3 for these magnitudes, and the
result is mathematically identical.)

SBUF layout: x streamed as [128, 512] tiles = 2 rows per tile, partition
p = r*64 + j (r = row parity, j = channel). The c-reduction lands in
y_coll[128, G]; its partition halves ARE the transposed-MLP operand
yT [j, col] for even/odd rows, so no on-chip transpose is ever needed.
"""

import os
import sys

import ml_dtypes
import numpy as np

for _p in ("/opt/trn_rl_repo",):
    if _p not in sys.path:
        sys.path.insert(0, _p)

from contextlib import ExitStack

from concourse import bacc, mybir, tile
from concourse.bass_utils import run_bass_kernel_spmd

N_CORES = 8
ROWS = 2048              # total B rows
C = 512
N = 64
P = 128
TILES = (ROWS // N_CORES) // 2   # 128 [128, 512] tiles per core, 2 rows each
G = 16                           # tiles per MLP chunk
FP = mybir.dt.float32
BF = mybir.dt.bfloat16   # streamed dtype for x / out: halves HBM traffic.
                         # rel err ~2.9e-3 vs the fp32 reference (gate 2e-2);
                         # DVE reduce accumulates in fp32 ALUs regardless.
SCALE = float(32 ** -0.5)
TPD = 8          # tiles (256 KiB each) per DMA transfer
HOST_PERM = True  # host pre-permutes shards so every DMA is contiguous

_CACHED = None
LAST_RESULTS = None  # BassKernelResults of the most recent kernel() call


def _build_module(
    tiles=TILES,
    g=G,
    repeat=1,
    tpd=TPD,
    store_engine="sync",
    xbufs=12,
    direct_scale=False,
    sv_engine="vector",
    sv_batch=True,
    mul_engine="scalar",
    mlp_bufs=2,
    host_perm=HOST_PERM,
    fine_tail=False,
):
    """repeat>1 wraps the streaming pass in an on-device For_i loop —
    used only for differential exec-time measurement (dispatch overhead
    cancels between two repeat counts).

    tpd = tiles per DMA: each load/store moves tpd*256KiB in one dma_start
    (3D access pattern [p, tpd, c]); bigger transfers amortize the per-DMA
    fixed cost. Loads issue on the SP HWDGE ring (nc.sync), stores on the
    ACT ring (nc.scalar) so the two streams don't share one FIFO."""
    nchunk = tiles // g
    assert g % tpd == 0
    nc = bacc.Bacc("TRN2", target_bir_lowering=False, debug=False)

    # host_perm: the host pre-permutes each shard to [tiles//tpd, P, tpd*C]
    # (group-major, partition-major) so every load/store is a fully
    # contiguous 2D AP — tpd*2KiB per partition per descriptor instead of
    # tpd separate 2KiB runs. The SBUF-side layout is identical.
    if host_perm:
        x_d = nc.dram_tensor("x", [tiles // tpd, P, tpd * C], BF, kind="ExternalInput")
    else:
        x_d = nc.dram_tensor("x", [tiles, P, C], BF, kind="ExternalInput")
    a_d = nc.dram_tensor("amat", [N, P], FP, kind="ExternalInput")
    be_d = nc.dram_tensor("bexp", [P, 1], FP, kind="ExternalInput")
    dg_d = nc.dram_tensor("daug", [P, N + 1], FP, kind="ExternalInput")
    b2_d = nc.dram_tensor("b2", [N, 1], FP, kind="ExternalInput")
    if host_perm:
        o_d = nc.dram_tensor("out", [tiles // tpd, P, tpd * C], BF, kind="ExternalOutput")
    else:
        o_d = nc.dram_tensor("out", [tiles, P, C], BF, kind="ExternalOutput")

    with tile.TileContext(nc) as tc, ExitStack() as ctx:
        const = ctx.enter_context(tc.tile_pool(name="const", bufs=1))
        xp = ctx.enter_context(
            tc.tile_pool(name="xp", bufs=xbufs or (2 * g // tpd))
        )
        yp = ctx.enter_context(tc.tile_pool(name="yp", bufs=mlp_bufs))
        sp = ctx.enter_context(tc.tile_pool(name="sp", bufs=mlp_bufs))
        svp = ctx.enter_context(tc.tile_pool(name="svp", bufs=2 * g))
        # 3 PSUM tags (f1/zs/rb) x bufs must fit 8 banks -> cap at 2
        pp = ctx.enter_context(
            tc.tile_pool(name="pp", bufs=min(mlp_bufs, 2), space="PSUM")
        )

        a_sb = const.tile([N, P], FP)
        nc.sync.dma_start(a_sb[:], a_d[:])
        be_sb = const.tile([P, 1], FP)
        nc.sync.dma_start(be_sb[:], be_d[:])
        dg_sb = const.tile([P, N + 1], FP)
        nc.sync.dma_start(dg_sb[:], dg_d[:])
        b2_sb = const.tile([N, 1], FP)
        nc.sync.dma_start(b2_sb[:], b2_d[:])
        ones_sb = const.tile([1, N], FP)
        nc.vector.memset(ones_sb[:], 1.0)

        loop_cm = tc.For_i(0, repeat, 1) if repeat > 1 else None
        if loop_cm is not None:
            loop_cm.__enter__()

        st_eng = {"scalar": nc.scalar, "sync": nc.sync, "gpsimd": nc.gpsimd}[
            store_engine
        ]
        for ch in range(nchunk):
            y_coll = yp.tile([P, g], FP)
            xts = []
            for i in range(0, g, tpd):
                t = ch * g + i
                xt = xp.tile([P, tpd * C], BF)
                xt3 = xt[:].rearrange("p (d c) -> p d c", d=tpd)
                if host_perm:
                    nc.sync.dma_start(xt[:], x_d[t // tpd])
                else:
                    nc.sync.dma_start(
                        xt3, x_d[t : t + tpd].rearrange("d p c -> p d c")
                    )
                nc.vector.reduce_sum(
                    y_coll[:, i : i + tpd], xt3, axis=mybir.AxisListType.X
                )
                xts.append(xt)

            # y_coll halves are yT for even/odd rows: pack to [64, 2g]
            y_all = sp.tile([N, 2 * g], FP)
            nc.vector.tensor_copy(y_all[:, 0:g], y_coll[0:N, :])
            nc.vector.tensor_copy(y_all[:, g : 2 * g], y_coll[N:P, :])

            f1 = pp.tile([P, 2 * g], FP)
            nc.tensor.matmul(f1[:], a_sb[:], y_all[:])
            e_sb = sp.tile([P, 2 * g], FP)
            nc.scalar.activation(
                e_sb[:], f1[:], mybir.ActivationFunctionType.Exp,
                bias=be_sb[:], scale=SCALE,
            )
            zs = pp.tile([N + 1, 2 * g], FP)
            nc.tensor.matmul(zs[:], dg_sb[:], e_sb[:])
            rs = sp.tile([1, 2 * g], FP)
            nc.vector.reciprocal(rs[:], zs[N : N + 1, :])
            rb = pp.tile([N, 2 * g], FP)
            nc.tensor.matmul(rb[:], ones_sb[:], rs[:])
            rb_sb = sp.tile([N, 2 * g], FP)
            nc.scalar.copy(rb_sb[:], rb[:])
            zn = sp.tile([N, 2 * g], FP)
            nc.vector.tensor_mul(zn[:], zs[0:N, :], rb_sb[:])
            y2 = sp.tile([N, 2 * g], FP)
            nc.scalar.activation(
                y2[:], zn[:], mybir.ActivationFunctionType.Sigmoid, bias=b2_sb[:]
            )

            svc = None
            if sv_batch and not direct_scale:
                # all g per-tile scale vectors assembled in two copies:
                # svc[(r,j), i] = y2[j, r*g + i]
                sv_eng = getattr(nc, sv_engine)
                svc = svp.tile([P, g], FP)
                sv_eng.tensor_copy(svc[0:N, :], y2[:, 0:g])
                sv_eng.tensor_copy(svc[N:P, :], y2[:, g : 2 * g])

            for i in range(0, g, tpd):
                t = ch * g + i
                xt = xts[i // tpd]
                for u in range(tpd):
                    col = xt[:, u * C : (u + 1) * C]
                    if mul_engine == "scalar" or (
                        mul_engine == "mixed" and (i // tpd) % 2 == 0
                    ):
                        mul_eng = nc.scalar
                    elif mul_engine == "vector":
                        mul_eng = nc.vector
                    else:
                        mul_eng = nc.gpsimd
                    if svc is not None:
                        if mul_eng is nc.scalar:
                            nc.scalar.activation(
                                col, col,
                                mybir.ActivationFunctionType.Copy,
                                scale=svc[:, i + u : i + u + 1],
                            )
                        else:
                            mul_eng.tensor_scalar_mul(
                                col, col, svc[:, i + u : i + u + 1]
                            )
                    elif direct_scale:
                        # two half-partition muls reading y2 columns as the
                        # per-partition scale directly (no sv assembly)
                        nc.scalar.activation(
                            col[0:N, :], col[0:N, :],
                            mybir.ActivationFunctionType.Copy,
                            scale=y2[:, i + u : i + u + 1],
                        )
                        nc.scalar.activation(
                            col[N:P, :], col[N:P, :],
                            mybir.ActivationFunctionType.Copy,
                            scale=y2[:, g + i + u : g + i + u + 1],
                        )
                    else:
                        sv_eng = getattr(nc, sv_engine)
                        sv = svp.tile([P, 1], FP)
                        sv_eng.tensor_copy(sv[0:N, :], y2[:, i + u : i + u + 1])
                        sv_eng.tensor_copy(
                            sv[N:P, :], y2[:, g + i + u : g + i + u + 1]
                        )
                        nc.scalar.activation(
                            col, col,
                            mybir.ActivationFunctionType.Copy,
                            scale=sv[:],
                        )
                if host_perm:
                    if fine_tail and ch == nchunk - 1:
                        # last chunk: stream stores out in 2-tile pieces as
                        # their muls land, shortening the serial kernel tail
                        for s0 in range(0, tpd, 2):
                            st_eng.dma_start(
                                o_d[t // tpd][:, s0 * C : (s0 + 2) * C],
                                xt[:, s0 * C : (s0 + 2) * C],
                            )
                    else:
                        st_eng.dma_start(o_d[t // tpd], xt[:])
                else:
                    st_eng.dma_start(
                        o_d[t : t + tpd].rearrange("d p c -> p d c"),
                        xt[:].rearrange("p (d c) -> p d c", d=tpd),
                    )

        if loop_cm is not None:
            loop_cm.__exit__(None, None, None)

    nc.compile()
    return nc


def _prep_weights(w1, b1, w2, b2, mb):
    w1 = np.asarray(w1, np.float64)
    b1 = np.asarray(b1, np.float64)
    w2 = np.asarray(w2, np.float64)
    b2 = np.asarray(b2, np.float64)
    mb = np.asarray(mb, np.float64)
    a = np.ascontiguousarray(((w1.T @ mb) / C).astype(np.float32))
    be = np.ascontiguousarray(((b1 @ mb) * SCALE).astype(np.float32).reshape(P, 1))
    dg = np.concatenate([(w2 @ mb).T, np.ones((P, 1))], axis=1)
    dg = np.ascontiguousarray(dg.astype(np.float32))
    b2c = np.ascontiguousarray(b2.astype(np.float32).reshape(N, 1))
    return a, be, dg, b2c


def kernel(x, w1, b1, w2, b2, mb):
    global _CACHED
    x = np.asarray(x, np.float32)
    b, Nn, Nwin, p, n, c = x.shape
    a, be, dg, b2c = _prep_weights(w1, b1, w2, b2, mb)

    if _CACHED is None:
        _CACHED = _build_module()
    nc = _CACHED

    xs = x.reshape(N_CORES, TILES, P, C).astype(ml_dtypes.bfloat16)
    if HOST_PERM:
        # group-major, partition-major packing: every on-device DMA becomes
        # one contiguous TPD*2KiB run per partition (see _build_module)
        xs = np.ascontiguousarray(
            xs.reshape(N_CORES, TILES // TPD, TPD, P, C).transpose(0, 1, 3, 2, 4)
        ).reshape(N_CORES, TILES // TPD, P, TPD * C)
    in_maps = [
        {"x": xs[i], "amat": a, "bexp": be, "daug": dg, "b2": b2c}
        for i in range(N_CORES)
    ]
    global LAST_RESULTS
    LAST_RESULTS = run_bass_kernel_spmd(
        nc, in_maps, core_ids=list(range(N_CORES)),
        trace=bool(os.environ.get("KERNEL_TRACE")),
    )
    res = LAST_RESULTS.results
    out = np.stack([r["out"] for r in res], axis=0)
    if HOST_PERM:
        out = np.ascontiguousarray(
            out.reshape(N_CORES, TILES // TPD, P, TPD, C).transpose(0, 1, 3, 2, 4)
        )
    return out.reshape(b, Nn, Nwin, p, n, c).astype(np.float32)


if __name__ == "__main__":
    xt = np.random.randn(2, 16, 16, 4, 64, 512).astype(np.float32)
    w1t = (np.random.randn(32, 64) * 0.1).astype(np.float32)
    b1t = (np.random.randn(32) * 0.1).astype(np.float32)
    w2t = (np.random.randn(64, 32) * 0.1).astype(np.float32)
    b2t = (np.random.randn(64) * 0.1).astype(np.float32)
    mbt = np.random.randn(32, 128).astype(np.float32)
    o = kernel(xt, w1t, b1t, w2t, b2t, mbt)
    print(o.shape, o.dtype)



# revision 13
# speedup vs baseline: 5.7667x; 5.7667x over previous
"""Trainium2 Bass kernel for a ChannelAttention module.

Reference computation (per row b of B = 2048 rows, each row is (n=64, c=512)):
    y  = mean_c x                      # (B, 64)
    lr = y @ w1.T + b1                 # (B, 32)
    f1 = lr @ mb                       # (B, 128)
    at = softmax(f1 / sqrt(32))        # (B, 128)
    y1 = at @ mb.T                     # (B, 32)
    y2 = sigmoid(y1 @ w2.T + b2)       # (B, 64)
    out = x * y2[..., None]

Memory-bound: 256 MiB in + 256 MiB out fp32. Strategy: data-parallel over
8 cores (256 rows each), single streaming pass per core, with x streamed in
BF16 both directions (host casts x -> bf16, device writes bf16, host upcasts;
rel err 2.3e-3 vs the 2e-2 gate) — halves HBM traffic to 16+16 MiB/core.

Measured TRN2 facts driving the config (see bench2.py differential timing):
  - DMA transfers on one HWDGE ring serialize completely: load-only 51.7us
    (325 GB/s) and store-only 48.9us, but load+store on the sync ring = the
    sum (106us). Stores therefore issue on the gpsimd SWDGE ring (slightly
    better than sync: 119 vs 126us full-kernel).
  - ACT runs 1 elem/cyc/lane dtype-independent -> 92us if it does all the
    scale-muls; DVE tensor_scalar packs bf16 4/cyc -> ~25us. Muls go to DVE.
  - DVE reduce runs ~1 elem/cyc (68us for all 16 transfers/iter); the
    c-reduction is split: 9 of 16 transfers reduce on ACT via
    activation(Copy, accum_out=...) per tile column, 7 on DVE reduce_sum,
    balancing both engines at ~55-60us, under the ~106us DMA wall.

The two inner linears fold host-side into two small fused matrices so the
on-chip MLP is:
    f1_raw = y_sum @ A          A = (w1.T @ mb) / 512          [64, 128]
    e      = exp(f1_raw*s + be) be = (b1 @ mb) * s, s=32^-0.5  [128, 1]
    [z|S]  = Daug.T @ e         Daug = [(w2 @ mb).T | ones]    [128, 65]
    y2     = sigmoid(z / S + b2)
(softmax max-subtraction is skipped: |f1*s| < ~3 for these magnitudes, and the
result is mathematically identical.)

SBUF layout: x streamed as [128, 512] bf16 tiles = 2 rows per tile, partition
p = r*64 + j (r = row parity, j = channel). The c-reduction lands in
y_coll[128, G] fp32; its partition halves ARE the transposed-MLP operand
yT [j, col] for even/odd rows, so no on-chip transpose is ever needed.
"""

import os
import sys

import ml_dtypes
import numpy as np

for _p in ("/opt/trn_rl_repo",):
    if _p not in sys.path:
        sys.path.insert(0, _p)

from contextlib import ExitStack

from concourse import bacc, mybir, tile
from concourse.bass_utils import run_bass_kernel_spmd

N_CORES = 8
ROWS = 2048              # total B rows
C = 512
N = 64
P = 128
TILES = (ROWS // N_CORES) // 2   # 128 [128, 512] tiles per core, 2 rows each
G = 16                           # tiles per MLP chunk
FP = mybir.dt.float32
BF = mybir.dt.bfloat16   # streamed dtype for x / out: halves HBM traffic.
                         # rel err ~2.9e-3 vs the fp32 reference (gate 2e-2);
                         # DVE reduce accumulates in fp32 ALUs regardless.
SCALE = float(32 ** -0.5)
TPD = 8          # tiles (256 KiB each) per DMA transfer
HOST_PERM = True  # host pre-permutes shards so every DMA is contiguous

_CACHED = None
LAST_RESULTS = None  # BassKernelResults of the most recent kernel() call


def _build_module(
    tiles=TILES,
    g=G,
    repeat=1,
    tpd=TPD,
    store_engine="gpsimd",
    xbufs=12,
    direct_scale=False,
    sv_engine="vector",
    sv_batch=True,
    mul_engine="vector",
    mlp_bufs=2,
    host_perm=HOST_PERM,
    fine_tail=False,
    reduce_act=9,
):
    """repeat>1 wraps the streaming pass in an on-device For_i loop —
    used only for differential exec-time measurement (dispatch overhead
    cancels between two repeat counts).

    tpd = tiles per DMA: each load/store moves tpd*256KiB in one dma_start
    (3D access pattern [p, tpd, c]); bigger transfers amortize the per-DMA
    fixed cost. Loads issue on the SP HWDGE ring (nc.sync), stores on the
    ACT ring (nc.scalar) so the two streams don't share one FIFO."""
    nchunk = tiles // g
    assert g % tpd == 0
    nc = bacc.Bacc("TRN2", target_bir_lowering=False, debug=False)

    # host_perm: the host pre-permutes each shard to [tiles//tpd, P, tpd*C]
    # (group-major, partition-major) so every load/store is a fully
    # contiguous 2D AP — tpd*2KiB per partition per descriptor instead of
    # tpd separate 2KiB runs. The SBUF-side layout is identical.
    if host_perm:
        x_d = nc.dram_tensor("x", [tiles // tpd, P, tpd * C], BF, kind="ExternalInput")
    else:
        x_d = nc.dram_tensor("x", [tiles, P, C], BF, kind="ExternalInput")
    a_d = nc.dram_tensor("amat", [N, P], FP, kind="ExternalInput")
    be_d = nc.dram_tensor("bexp", [P, 1], FP, kind="ExternalInput")
    dg_d = nc.dram_tensor("daug", [P, N + 1], FP, kind="ExternalInput")
    b2_d = nc.dram_tensor("b2", [N, 1], FP, kind="ExternalInput")
    if host_perm:
        o_d = nc.dram_tensor("out", [tiles // tpd, P, tpd * C], BF, kind="ExternalOutput")
    else:
        o_d = nc.dram_tensor("out", [tiles, P, C], BF, kind="ExternalOutput")

    with tile.TileContext(nc) as tc, ExitStack() as ctx:
        const = ctx.enter_context(tc.tile_pool(name="const", bufs=1))
        xp = ctx.enter_context(
            tc.tile_pool(name="xp", bufs=xbufs or (2 * g // tpd))
        )
        trp = ctx.enter_context(tc.tile_pool(name="trp", bufs=2))
        yp = ctx.enter_context(tc.tile_pool(name="yp", bufs=mlp_bufs))
        sp = ctx.enter_context(tc.tile_pool(name="sp", bufs=mlp_bufs))
        svp = ctx.enter_context(tc.tile_pool(name="svp", bufs=2 * g))
        # 3 PSUM tags (f1/zs/rb) x bufs must fit 8 banks -> cap at 2
        pp = ctx.enter_context(
            tc.tile_pool(name="pp", bufs=min(mlp_bufs, 2), space="PSUM")
        )

        a_sb = const.tile([N, P], FP)
        nc.sync.dma_start(a_sb[:], a_d[:])
        be_sb = const.tile([P, 1], FP)
        nc.sync.dma_start(be_sb[:], be_d[:])
        dg_sb = const.tile([P, N + 1], FP)
        nc.sync.dma_start(dg_sb[:], dg_d[:])
        b2_sb = const.tile([N, 1], FP)
        nc.sync.dma_start(b2_sb[:], b2_d[:])
        ones_sb = const.tile([1, N], FP)
        nc.vector.memset(ones_sb[:], 1.0)

        loop_cm = tc.For_i(0, repeat, 1) if repeat > 1 else None
        if loop_cm is not None:
            loop_cm.__enter__()

        st_eng = {"scalar": nc.scalar, "sync": nc.sync, "gpsimd": nc.gpsimd}[
            store_engine
        ]
        ntr_total = tiles // tpd
        # evenly-spaced transfer indices whose c-reduction runs on the ACT
        # engine (activation Copy + accum_out, one instr per tile column)
        # instead of DVE reduce_sum — balances the two engines' busy time
        act_set = (
            {round(j * ntr_total / reduce_act) for j in range(reduce_act)}
            if reduce_act
            else set()
        )
        for ch in range(nchunk):
            y_coll = yp.tile([P, g], FP)
            xts = []
            for i in range(0, g, tpd):
                t = ch * g + i
                xt = xp.tile([P, tpd * C], BF)
                xt3 = xt[:].rearrange("p (d c) -> p d c", d=tpd)
                if host_perm:
                    nc.sync.dma_start(xt[:], x_d[t // tpd])
                else:
                    nc.sync.dma_start(
                        xt3, x_d[t : t + tpd].rearrange("d p c -> p d c")
                    )
                if t // tpd in act_set:
                    trash = trp.tile([P, C], BF)
                    for u in range(tpd):
                        nc.scalar.activation(
                            trash[:],
                            xt[:, u * C : (u + 1) * C],
                            mybir.ActivationFunctionType.Copy,
                            accum_out=y_coll[:, i + u : i + u + 1],
                        )
                else:
                    nc.vector.reduce_sum(
                        y_coll[:, i : i + tpd], xt3, axis=mybir.AxisListType.X
                    )
                xts.append(xt)

            # y_coll halves are yT for even/odd rows: pack to [64, 2g]
            y_all = sp.tile([N, 2 * g], FP)
            nc.vector.tensor_copy(y_all[:, 0:g], y_coll[0:N, :])
            nc.vector.tensor_copy(y_all[:, g : 2 * g], y_coll[N:P, :])

            f1 = pp.tile([P, 2 * g], FP)
            nc.tensor.matmul(f1[:], a_sb[:], y_all[:])
            e_sb = sp.tile([P, 2 * g], FP)
            nc.scalar.activation(
                e_sb[:], f1[:], mybir.ActivationFunctionType.Exp,
                bias=be_sb[:], scale=SCALE,
            )
            zs = pp.tile([N + 1, 2 * g], FP)
            nc.tensor.matmul(zs[:], dg_sb[:], e_sb[:])
            rs = sp.tile([1, 2 * g], FP)
            nc.vector.reciprocal(rs[:], zs[N : N + 1, :])
            rb = pp.tile([N, 2 * g], FP)
            nc.tensor.matmul(rb[:], ones_sb[:], rs[:])
            rb_sb = sp.tile([N, 2 * g], FP)
            nc.scalar.copy(rb_sb[:], rb[:])
            zn = sp.tile([N, 2 * g], FP)
            nc.vector.tensor_mul(zn[:], zs[0:N, :], rb_sb[:])
            y2 = sp.tile([N, 2 * g], FP)
            nc.scalar.activation(
                y2[:], zn[:], mybir.ActivationFunctionType.Sigmoid, bias=b2_sb[:]
            )

            svc = None
            if sv_batch and not direct_scale:
                # all g per-tile scale vectors assembled in two copies:
                # svc[(r,j), i] = y2[j, r*g + i]
                sv_eng = getattr(nc, sv_engine)
                svc = svp.tile([P, g], FP)
                sv_eng.tensor_copy(svc[0:N, :], y2[:, 0:g])
                sv_eng.tensor_copy(svc[N:P, :], y2[:, g : 2 * g])

            for i in range(0, g, tpd):
                t = ch * g + i
                xt = xts[i // tpd]
                for u in range(tpd):
                    col = xt[:, u * C : (u + 1) * C]
                    if mul_engine == "scalar" or (
                        mul_engine == "mixed" and (i // tpd) % 2 == 0
                    ):
                        mul_eng = nc.scalar
                    elif mul_engine == "vector":
                        mul_eng = nc.vector
                    else:
                        mul_eng = nc.gpsimd
                    if svc is not None:
                        if mul_eng is nc.scalar:
                            nc.scalar.activation(
                                col, col,
                                mybir.ActivationFunctionType.Copy,
                                scale=svc[:, i + u : i + u + 1],
                            )
                        else:
                            mul_eng.tensor_scalar_mul(
                                col, col, svc[:, i + u : i + u + 1]
                            )
                    elif direct_scale:
                        # two half-partition muls reading y2 columns as the
                        # per-partition scale directly (no sv assembly)
                        nc.scalar.activation(
                            col[0:N, :], col[0:N, :],
                            mybir.ActivationFunctionType.Copy,
                            scale=y2[:, i + u : i + u + 1],
                        )
                        nc.scalar.activation(
                            col[N:P, :], col[N:P, :],
                            mybir.ActivationFunctionType.Copy,
                            scale=y2[:, g + i + u : g + i + u + 1],
                        )
                    else:
                        sv_eng = getattr(nc, sv_engine)
                        sv = svp.tile([P, 1], FP)
                        sv_eng.tensor_copy(sv[0:N, :], y2[:, i + u : i + u + 1])
                        sv_eng.tensor_copy(
                            sv[N:P, :], y2[:, g + i + u : g + i + u + 1]
                        )
                        nc.scalar.activation(
                            col, col,
                            mybir.ActivationFunctionType.Copy,
                            scale=sv[:],
                        )
                if host_perm:
                    if fine_tail and ch == nchunk - 1:
                        # last chunk: stream stores out in 2-tile pieces as
                        # their muls land, shortening the serial kernel tail
                        for s0 in range(0, tpd, 2):
                            st_eng.dma_start(
                                o_d[t // tpd][:, s0 * C : (s0 + 2) * C],
                                xt[:, s0 * C : (s0 + 2) * C],
                            )
                    else:
                        st_eng.dma_start(o_d[t // tpd], xt[:])
                else:
                    st_eng.dma_start(
                        o_d[t : t + tpd].rearrange("d p c -> p d c"),
                        xt[:].rearrange("p (d c) -> p d c", d=tpd),
                    )

        if loop_cm is not None:
            loop_cm.__exit__(None, None, None)

    nc.compile()
    return nc


def _prep_weights(w1, b1, w2, b2, mb):
    w1 = np.asarray(w1, np.float64)
    b1 = np.asarray(b1, np.float64)
    w2 = np.asarray(w2, np.float64)
    b2 = np.asarray(b2, np.float64)
    mb = np.asarray(mb, np.float64)
    a = np.ascontiguousarray(((w1.T @ mb) / C).astype(np.float32))
    be = np.ascontiguousarray(((b1 @ mb) * SCALE).astype(np.float32).reshape(P, 1))
    dg = np.concatenate([(w2 @ mb).T, np.ones((P, 1))], axis=1)
    dg = np.ascontiguousarray(dg.astype(np.float32))
    b2c = np.ascontiguousarray(b2.astype(np.float32).reshape(N, 1))
    return a, be, dg, b2c


def kernel(x, w1, b1, w2, b2, mb):
    global _CACHED
    x = np.asarray(x, np.float32)
    b, Nn, Nwin, p, n, c = x.shape
    a, be, dg, b2c = _prep_weights(w1, b1, w2, b2, mb)

    if _CACHED is None:
        _CACHED = _build_module()
    nc = _CACHED

    xs = x.reshape(N_CORES, TILES, P, C).astype(ml_dtypes.bfloat16)
    if HOST_PERM:
        # group-major, partition-major packing: every on-device DMA becomes
        # one contiguous TPD*2KiB run per partition (see _build_module)
        xs = np.ascontiguousarray(
            xs.reshape(N_CORES, TILES // TPD, TPD, P, C).transpose(0, 1, 3, 2, 4)
        ).reshape(N_CORES, TILES // TPD, P, TPD * C)
    in_maps = [
        {"x": xs[i], "amat": a, "bexp": be, "daug": dg, "b2": b2c}
        for i in range(N_CORES)
    ]
    global LAST_RESULTS
    LAST_RESULTS = run_bass_kernel_spmd(
        nc, in_maps, core_ids=list(range(N_CORES)),
        trace=bool(os.environ.get("KERNEL_TRACE")),
    )
    res = LAST_RESULTS.results
    out = np.stack([r["out"] for r in res], axis=0)
    if HOST_PERM:
        out = np.ascontiguousarray(
            out.reshape(N_CORES, TILES // TPD, P, TPD, C).transpose(0, 1, 3, 2, 4)
        )
    return out.reshape(b, Nn, Nwin, p, n, c).astype(np.float32)


if __name__ == "__main__":
    xt = np.random.randn(2, 16, 16, 4, 64, 512).astype(np.float32)
    w1t = (np.random.randn(32, 64) * 0.1).astype(np.float32)
    b1t = (np.random.randn(32) * 0.1).astype(np.float32)
    w2t = (np.random.randn(64, 32) * 0.1).astype(np.float32)
    b2t = (np.random.randn(64) * 0.1).astype(np.float32)
    mbt = np.random.randn(32, 128).astype(np.float32)
    o = kernel(xt, w1t, b1t, w2t, b2t, mbt)
    print(o.shape, o.dtype)



# revision 15
# speedup vs baseline: 6.0675x; 1.0522x over previous
"""Trainium2 Bass kernel for a ChannelAttention module.

Reference computation (per row b of B = 2048 rows, each row is (n=64, c=512)):
    y  = mean_c x                      # (B, 64)
    lr = y @ w1.T + b1                 # (B, 32)
    f1 = lr @ mb                       # (B, 128)
    at = softmax(f1 / sqrt(32))        # (B, 128)
    y1 = at @ mb.T                     # (B, 32)
    y2 = sigmoid(y1 @ w2.T + b2)       # (B, 64)
    out = x * y2[..., None]

Memory-bound: 256 MiB in + 256 MiB out fp32. Strategy: data-parallel over
8 cores (256 rows each), single streaming pass per core, with x streamed in
BF16 both directions (host casts x -> bf16, device writes bf16, host upcasts;
rel err 2.3e-3 vs the 2e-2 gate) — halves HBM traffic to 16+16 MiB/core.

Measured TRN2 facts driving the config (see bench2.py differential timing):
  - DMA transfers on one HWDGE ring serialize completely: load-only 51.7us
    (325 GB/s) and store-only 48.9us, but load+store on the sync ring = the
    sum (106us). Stores therefore issue on the gpsimd SWDGE ring (slightly
    better than sync: 119 vs 126us full-kernel).
  - ACT runs 1 elem/cyc/lane dtype-independent -> 92us if it does all the
    scale-muls; DVE tensor_scalar packs bf16 4/cyc -> ~25us. Muls go to DVE.
  - DVE reduce runs ~1 elem/cyc (68us for all 16 transfers/iter); the
    c-reduction is split: 6 of 16 transfers reduce on ACT via
    activation(Copy, accum_out=...) per tile column, 10 on DVE reduce_sum
    (measured best split), keeping both engines under the ~106us DMA wall.

The two inner linears fold host-side into two small fused matrices so the
on-chip MLP is:
    f1_raw = y_sum @ A          A = (w1.T @ mb) / 512          [64, 128]
    e      = exp(f1_raw*s + be) be = (b1 @ mb) * s, s=32^-0.5  [128, 1]
    [z|S]  = Daug.T @ e         Daug = [(w2 @ mb).T | ones]    [128, 65]
    y2     = sigmoid(z / S + b2)
(softmax max-subtraction is skipped: |f1*s| < ~3 for these magnitudes, and the
result is mathematically identical.)

SBUF layout: x streamed as [128, 512] bf16 tiles = 2 rows per tile, partition
p = r*64 + j (r = row parity, j = channel). The c-reduction lands in
y_coll[128, G] fp32; its partition halves ARE the transposed-MLP operand
yT [j, col] for even/odd rows, so no on-chip transpose is ever needed.
"""

import os
import sys

import ml_dtypes
import numpy as np

for _p in ("/opt/trn_rl_repo",):
    if _p not in sys.path:
        sys.path.insert(0, _p)

from contextlib import ExitStack

from concourse import bacc, mybir, tile
from concourse.bass_utils import run_bass_kernel_spmd

N_CORES = 8
ROWS = 2048              # total B rows
C = 512
N = 64
P = 128
TILES = (ROWS // N_CORES) // 2   # 128 [128, 512] tiles per core, 2 rows each
G = 16                           # tiles per MLP chunk
FP = mybir.dt.float32
BF = mybir.dt.bfloat16   # streamed dtype for x / out: halves HBM traffic.
                         # rel err ~2.9e-3 vs the fp32 reference (gate 2e-2);
                         # DVE reduce accumulates in fp32 ALUs regardless.
SCALE = float(32 ** -0.5)
TPD = 8          # tiles (256 KiB each) per DMA transfer
HOST_PERM = True  # host pre-permutes shards so every DMA is contiguous

_CACHED = None
LAST_RESULTS = None  # BassKernelResults of the most recent kernel() call


def _build_module(
    tiles=TILES,
    g=G,
    repeat=1,
    tpd=TPD,
    store_engine="gpsimd",
    xbufs=12,
    direct_scale=False,
    sv_engine="vector",
    sv_batch=True,
    mul_engine="vector",
    mlp_bufs=2,
    host_perm=HOST_PERM,
    fine_tail=False,
    reduce_act=6,
):
    """repeat>1 wraps the streaming pass in an on-device For_i loop —
    used only for differential exec-time measurement (dispatch overhead
    cancels between two repeat counts).

    tpd = tiles per DMA: each load/store moves tpd*256KiB in one dma_start
    (3D access pattern [p, tpd, c]); bigger transfers amortize the per-DMA
    fixed cost. Loads issue on the SP HWDGE ring (nc.sync), stores on the
    ACT ring (nc.scalar) so the two streams don't share one FIFO."""
    nchunk = tiles // g
    assert g % tpd == 0
    nc = bacc.Bacc("TRN2", target_bir_lowering=False, debug=False)

    # host_perm: the host pre-permutes each shard to [tiles//tpd, P, tpd*C]
    # (group-major, partition-major) so every load/store is a fully
    # contiguous 2D AP — tpd*2KiB per partition per descriptor instead of
    # tpd separate 2KiB runs. The SBUF-side layout is identical.
    if host_perm:
        x_d = nc.dram_tensor("x", [tiles // tpd, P, tpd * C], BF, kind="ExternalInput")
    else:
        x_d = nc.dram_tensor("x", [tiles, P, C], BF, kind="ExternalInput")
    a_d = nc.dram_tensor("amat", [N, P], FP, kind="ExternalInput")
    be_d = nc.dram_tensor("bexp", [P, 1], FP, kind="ExternalInput")
    dg_d = nc.dram_tensor("daug", [P, N + 1], FP, kind="ExternalInput")
    b2_d = nc.dram_tensor("b2", [N, 1], FP, kind="ExternalInput")
    if host_perm:
        o_d = nc.dram_tensor("out", [tiles // tpd, P, tpd * C], BF, kind="ExternalOutput")
    else:
        o_d = nc.dram_tensor("out", [tiles, P, C], BF, kind="ExternalOutput")

    with tile.TileContext(nc) as tc, ExitStack() as ctx:
        const = ctx.enter_context(tc.tile_pool(name="const", bufs=1))
        xp = ctx.enter_context(
            tc.tile_pool(name="xp", bufs=xbufs or (2 * g // tpd))
        )
        trp = ctx.enter_context(tc.tile_pool(name="trp", bufs=2))
        yp = ctx.enter_context(tc.tile_pool(name="yp", bufs=mlp_bufs))
        sp = ctx.enter_context(tc.tile_pool(name="sp", bufs=mlp_bufs))
        svp = ctx.enter_context(tc.tile_pool(name="svp", bufs=2 * g))
        # 3 PSUM tags (f1/zs/rb) x bufs must fit 8 banks -> cap at 2
        pp = ctx.enter_context(
            tc.tile_pool(name="pp", bufs=min(mlp_bufs, 2), space="PSUM")
        )

        a_sb = const.tile([N, P], FP)
        nc.sync.dma_start(a_sb[:], a_d[:])
        be_sb = const.tile([P, 1], FP)
        nc.sync.dma_start(be_sb[:], be_d[:])
        dg_sb = const.tile([P, N + 1], FP)
        nc.sync.dma_start(dg_sb[:], dg_d[:])
        b2_sb = const.tile([N, 1], FP)
        nc.sync.dma_start(b2_sb[:], b2_d[:])
        ones_sb = const.tile([1, N], FP)
        nc.vector.memset(ones_sb[:], 1.0)

        loop_cm = tc.For_i(0, repeat, 1) if repeat > 1 else None
        if loop_cm is not None:
            loop_cm.__enter__()

        st_eng = {"scalar": nc.scalar, "sync": nc.sync, "gpsimd": nc.gpsimd}[
            store_engine
        ]
        ntr_total = tiles // tpd
        # evenly-spaced transfer indices whose c-reduction runs on the ACT
        # engine (activation Copy + accum_out, one instr per tile column)
        # instead of DVE reduce_sum — balances the two engines' busy time
        act_set = (
            {round(j * ntr_total / reduce_act) for j in range(reduce_act)}
            if reduce_act
            else set()
        )
        for ch in range(nchunk):
            y_coll = yp.tile([P, g], FP)
            xts = []
            for i in range(0, g, tpd):
                t = ch * g + i
                xt = xp.tile([P, tpd * C], BF)
                xt3 = xt[:].rearrange("p (d c) -> p d c", d=tpd)
                if host_perm:
                    nc.sync.dma_start(xt[:], x_d[t // tpd])
                else:
                    nc.sync.dma_start(
                        xt3, x_d[t : t + tpd].rearrange("d p c -> p d c")
                    )
                if t // tpd in act_set:
                    trash = trp.tile([P, C], BF)
                    for u in range(tpd):
                        nc.scalar.activation(
                            trash[:],
                            xt[:, u * C : (u + 1) * C],
                            mybir.ActivationFunctionType.Copy,
                            accum_out=y_coll[:, i + u : i + u + 1],
                        )
                else:
                    nc.vector.reduce_sum(
                        y_coll[:, i : i + tpd], xt3, axis=mybir.AxisListType.X
                    )
                xts.append(xt)

            # y_coll halves are yT for even/odd rows: pack to [64, 2g]
            y_all = sp.tile([N, 2 * g], FP)
            nc.vector.tensor_copy(y_all[:, 0:g], y_coll[0:N, :])
            nc.vector.tensor_copy(y_all[:, g : 2 * g], y_coll[N:P, :])

            f1 = pp.tile([P, 2 * g], FP)
            nc.tensor.matmul(f1[:], a_sb[:], y_all[:])
            e_sb = sp.tile([P, 2 * g], FP)
            nc.scalar.activation(
                e_sb[:], f1[:], mybir.ActivationFunctionType.Exp,
                bias=be_sb[:], scale=SCALE,
            )
            zs = pp.tile([N + 1, 2 * g], FP)
            nc.tensor.matmul(zs[:], dg_sb[:], e_sb[:])
            rs = sp.tile([1, 2 * g], FP)
            nc.vector.reciprocal(rs[:], zs[N : N + 1, :])
            rb = pp.tile([N, 2 * g], FP)
            nc.tensor.matmul(rb[:], ones_sb[:], rs[:])
            rb_sb = sp.tile([N, 2 * g], FP)
            nc.scalar.copy(rb_sb[:], rb[:])
            zn = sp.tile([N, 2 * g], FP)
            nc.vector.tensor_mul(zn[:], zs[0:N, :], rb_sb[:])
            y2 = sp.tile([N, 2 * g], FP)
            nc.scalar.activation(
                y2[:], zn[:], mybir.ActivationFunctionType.Sigmoid, bias=b2_sb[:]
            )

            svc = None
            if sv_batch and not direct_scale:
                # all g per-tile scale vectors assembled in two copies:
                # svc[(r,j), i] = y2[j, r*g + i]
                sv_eng = getattr(nc, sv_engine)
                svc = svp.tile([P, g], FP)
                sv_eng.tensor_copy(svc[0:N, :], y2[:, 0:g])
                sv_eng.tensor_copy(svc[N:P, :], y2[:, g : 2 * g])

            for i in range(0, g, tpd):
                t = ch * g + i
                xt = xts[i // tpd]
                for u in range(tpd):
                    col = xt[:, u * C : (u + 1) * C]
                    if mul_engine == "scalar" or (
                        mul_engine == "mixed" and (i // tpd) % 2 == 0
                    ):
                        mul_eng = nc.scalar
                    elif mul_engine == "vector":
                        mul_eng = nc.vector
                    else:
                        mul_eng = nc.gpsimd
                    if svc is not None:
                        if mul_eng is nc.scalar:
                            nc.scalar.activation(
                                col, col,
                                mybir.ActivationFunctionType.Copy,
                                scale=svc[:, i + u : i + u + 1],
                            )
                        else:
                            mul_eng.tensor_scalar_mul(
                                col, col, svc[:, i + u : i + u + 1]
                            )
                    elif direct_scale:
                        # two half-partition muls reading y2 columns as the
                        # per-partition scale directly (no sv assembly)
                        nc.scalar.activation(
                            col[0:N, :], col[0:N, :],
                            mybir.ActivationFunctionType.Copy,
                            scale=y2[:, i + u : i + u + 1],
                        )
                        nc.scalar.activation(
                            col[N:P, :], col[N:P, :],
                            mybir.ActivationFunctionType.Copy,
                            scale=y2[:, g + i + u : g + i + u + 1],
                        )
                    else:
                        sv_eng = getattr(nc, sv_engine)
                        sv = svp.tile([P, 1], FP)
                        sv_eng.tensor_copy(sv[0:N, :], y2[:, i + u : i + u + 1])
                        sv_eng.tensor_copy(
                            sv[N:P, :], y2[:, g + i + u : g + i + u + 1]
                        )
                        nc.scalar.activation(
                            col, col,
                            mybir.ActivationFunctionType.Copy,
                            scale=sv[:],
                        )
                if host_perm:
                    if fine_tail and ch == nchunk - 1:
                        # last chunk: stream stores out in 2-tile pieces as
                        # their muls land, shortening the serial kernel tail
                        for s0 in range(0, tpd, 2):
                            st_eng.dma_start(
                                o_d[t // tpd][:, s0 * C : (s0 + 2) * C],
                                xt[:, s0 * C : (s0 + 2) * C],
                            )
                    else:
                        st_eng.dma_start(o_d[t // tpd], xt[:])
                else:
                    st_eng.dma_start(
                        o_d[t : t + tpd].rearrange("d p c -> p d c"),
                        xt[:].rearrange("p (d c) -> p d c", d=tpd),
                    )

        if loop_cm is not None:
            loop_cm.__exit__(None, None, None)

    nc.compile()
    return nc


def _prep_weights(w1, b1, w2, b2, mb):
    w1 = np.asarray(w1, np.float64)
    b1 = np.asarray(b1, np.float64)
    w2 = np.asarray(w2, np.float64)
    b2 = np.asarray(b2, np.float64)
    mb = np.asarray(mb, np.float64)
    a = np.ascontiguousarray(((w1.T @ mb) / C).astype(np.float32))
    be = np.ascontiguousarray(((b1 @ mb) * SCALE).astype(np.float32).reshape(P, 1))
    dg = np.concatenate([(w2 @ mb).T, np.ones((P, 1))], axis=1)
    dg = np.ascontiguousarray(dg.astype(np.float32))
    b2c = np.ascontiguousarray(b2.astype(np.float32).reshape(N, 1))
    return a, be, dg, b2c


def kernel(x, w1, b1, w2, b2, mb):
    global _CACHED
    x = np.asarray(x, np.float32)
    b, Nn, Nwin, p, n, c = x.shape
    a, be, dg, b2c = _prep_weights(w1, b1, w2, b2, mb)

    if _CACHED is None:
        _CACHED = _build_module()
    nc = _CACHED

    xs = x.reshape(N_CORES, TILES, P, C).astype(ml_dtypes.bfloat16)
    if HOST_PERM:
        # group-major, partition-major packing: every on-device DMA becomes
        # one contiguous TPD*2KiB run per partition (see _build_module)
        xs = np.ascontiguousarray(
            xs.reshape(N_CORES, TILES // TPD, TPD, P, C).transpose(0, 1, 3, 2, 4)
        ).reshape(N_CORES, TILES // TPD, P, TPD * C)
    in_maps = [
        {"x": xs[i], "amat": a, "bexp": be, "daug": dg, "b2": b2c}
        for i in range(N_CORES)
    ]
    global LAST_RESULTS
    LAST_RESULTS = run_bass_kernel_spmd(
        nc, in_maps, core_ids=list(range(N_CORES)),
        trace=bool(os.environ.get("KERNEL_TRACE")),
    )
    res = LAST_RESULTS.results
    out = np.stack([r["out"] for r in res], axis=0)
    if HOST_PERM:
        out = np.ascontiguousarray(
            out.reshape(N_CORES, TILES // TPD, P, TPD, C).transpose(0, 1, 3, 2, 4)
        )
    return out.reshape(b, Nn, Nwin, p, n, c).astype(np.float32)


if __name__ == "__main__":
    xt = np.random.randn(2, 16, 16, 4, 64, 512).astype(np.float32)
    w1t = (np.random.randn(32, 64) * 0.1).astype(np.float32)
    b1t = (np.random.randn(32) * 0.1).astype(np.float32)
    w2t = (np.random.randn(64, 32) * 0.1).astype(np.float32)
    b2t = (np.random.randn(64) * 0.1).astype(np.float32)
    mbt = np.random.randn(32, 128).astype(np.float32)
    o = kernel(xt, w1t, b1t, w2t, b2t, mbt)
    print(o.shape, o.dtype)

